# revision 9
# baseline (speedup 1.0000x reference)
"""Trainium2 Bass kernel for nn_GATLayer_58291296141986.

Math: the reference computes
    xt = (x @ W.T).reshape(B, N, H, D)            # B=32, N=10, H=8, D=8
    out[b,n,h,m] = relu(sum_k xt[b,n,h,k] * adj[b,n,m])
adj does not depend on k, so sum_k xt[b,n,h,k] = x[b,n,:] @ Wsum[h,:]
with Wsum[h] = sum_d W[h*8+d].  The whole problem collapses to
    s = x2 @ Wsum.T        # (320, 65536) @ (65536, 8)
    out[t, h*10+m] = relu(s[t,h] * adj[t,m])
which is memory-bound on reading x (84MB) + W (17MB).

Sharding: tensor-parallel over in_dim (k).  Each of the 8 cores reads a
disjoint 8192-wide k-slice of x (10.5MB) and W (2MB) and accumulates a
partial s64^T = W_slice @ x_slice^T of shape (64, 320) -- every input
byte is read exactly once across the chip (~12.6MB/core, the memory
roofline).  A tail selector-matmul folds the d dimension (rows h*8+d ->
head h), so each core outputs an (8, 320) partial of s.  The cross-core
reduction of those 10KB partials happens in a second, tiny SPMD launch:
the host hands core h the 8 partial rows of head h (pure data movement),
a ones-matmul sums them and replicates the result onto 10 PSUM
partitions, and one fused scalar_tensor_tensor computes relu(s)*adj^T
(valid since adj >= 0, so relu(s*adj) = relu(s)*adj).  Core h thus
produces the 10 output columns of head h for all 320 tokens and the
host concatenates the 8 head slices.  (A single-launch variant with an
on-device AllToAll was measured ~30us slower: the collectives
firmware's entry barrier alone costs ~60us on this runtime; and a
minimal-kernel probe measured ~12.6us of fixed NRT pre/postamble per
launch, so the fold launch is already near its floor.)

Wire scheduling (profiled, not guessed): the two HWDGE rings (SP and
ACT) share the 16 SDMA engines with per-PACKET round-robin, so the
instantaneous bandwidth split is proportional to packet (=partition
line) size -- a 10KB-line stream steals 2.5x from a concurrent
4KB-line stream.  The ACT ring also starts ~2.2us late (runtime
activation-table loads).  So: W moves as two 8KB-line halves (near the
xs 10KB lines; 2KB lines measured ~35% slower), one half per ring,
ahead of the xs chunks; the 8 xs chunks alternate rings, with chunk 7
split 3js/5js across the rings so the SP ring carries ~0.5MB more to
compensate the ACT handicap and both rings drain together.  The raw W
slices are the stationary operand (no DVE reduce anywhere), and the
tiny d-fold selector rides as 8 extra columns of the W tensor (a
separate 32B-line DMA measurably starved the ACT ring).

HAM warming: the PE's activity monitor keeps the array at 1.2GHz unless
it has been busy ~3.4us continuously, and chunked matmul bursts never
warm it -- profiled cold matmuls at 468ns vs 309ns warm made the PE
fall behind the DMA stream and trail ~5us past the last byte.  Dummy
full-width (N=320) filler matmuls into a scratch PSUM bank prime the
array before the first chunk and pad the idle between chunk bursts.

Device layout trick: the PE contracts over the partition axis, but x in
DRAM is token-major.  The host pre-swizzles each core's x slice to
    xs[p, j*320 + t] = x2[t, c*8192 + p*64 + j]   (p in 0..128, j in 0..64)
so one matmul per j (lhsT = W slice (128,64), rhs = xs slice (128,320))
accumulates s64^T over 64 PSUM-accumulated matmuls.  W is pre-swizzled
to ws[p, (j*8+h)*8+d] = W[h*8+d, c*8192 + p*64 + j] so each j's lhsT is
a contiguous (128, 64) slice.  Matmul operands are float32r: same fp32
bytes, single-pass PE matmul at 1 cycle/row (plain fp32 is 4
cycles/row), costing ~1e-4 relative error.
"""

import numpy as np

import concourse.bass as bass
import concourse.mybir as mybir
import concourse.tile as tile
from concourse import bacc
from concourse.bass_utils import run_bass_kernel_spmd

B, NN, IN_DIM, OUT_DIM, HEADS = 32, 10, 65536, 64, 8
NCORES = 8
T = B * NN                 # 320 tokens
KS = IN_DIM // NCORES      # 8192 contraction slice per core
JW = KS // 128             # 64 j-steps per core
NCHUNK = 8                 # xs DMA chunks
JC = JW // NCHUNK          # j-steps per chunk
WHC = JW * OUT_DIM // 2    # ws columns per half (2048 = 8KB/partition)
J7A = 3                    # js of chunk 7 carried by the SP ring
PRIME_FILL = 12            # HAM-priming fillers before chunk 0 (cold ~450ns each)
TOPPER = {0: 6, 2: 4, 4: 4}  # idle-padding fillers after these chunk bursts
F32 = mybir.dt.float32
F32R = mybir.dt.float32r


def build_main():
    """Launch 1: per-core partial s^T = fold_d((W k-slice) @ (x k-slice)^T)."""
    nc = bacc.Bacc("TRN2", debug=False, num_devices=NCORES, target_bir_lowering=False)

    xs_d = nc.dram_tensor("xs", [128, JW * T], F32R, kind="ExternalInput").ap()
    # last 8 columns = the d-fold selector (padded to 128 rows)
    ws_d = nc.dram_tensor(
        "ws", [128, JW * OUT_DIM + HEADS], F32R, kind="ExternalInput"
    ).ap()
    part_d = nc.dram_tensor("part", [HEADS, T], F32, kind="ExternalOutput").ap()

    with tile.TileContext(nc) as tc:
        with (
            tc.tile_pool(name="xp", bufs=NCHUNK) as xp,
            tc.tile_pool(name="wp", bufs=1) as wp,
            tc.tile_pool(name="aux", bufs=1) as aux,
            tc.tile_pool(name="pp", bufs=1, space="PSUM") as pp,
        ):
            # W halves, one per ring, ahead of the xs chunks; half b covers
            # js [32b, 32b+32) = xs chunks 4b..4b+3; the scalar half also
            # carries the selector columns
            wst0 = wp.tile([128, WHC], F32R, name="wst0", tag="wst0")
            nc.sync.dma_start(wst0[:], ws_d[:, :WHC])
            wst1 = wp.tile([128, WHC + HEADS], F32R, name="wst1", tag="wst1")
            nc.scalar.dma_start(wst1[:], ws_d[:, WHC:])
            wsts = [wst0, wst1]

            psum_s = pp.tile([OUT_DIM, T], F32, name="psum_s")
            psum_junk = pp.tile([OUT_DIM, T], F32, name="psum_junk")

            def filler(n):
                # garbage matmuls on the resident W half: keep the PE busy
                # so HAM holds the array at 2.4GHz (values unused; full
                # N=320 streams so the activity monitor registers them)
                for _ in range(n):
                    nc.tensor.matmul(
                        psum_junk[:],
                        wst0[:, :OUT_DIM],
                        wst0[:, :T],
                        start=True,
                        stop=True,
                    )

            filler(PRIME_FILL)

            # xs chunks alternate rings; chunk 7 is split 3js/5js across
            # the rings to rebalance the ACT ring's late start
            for jc in range(NCHUNK):
                xt = xp.tile([128, JC * T], F32R, name=f"xt{jc}", tag="xt")
                if jc < NCHUNK - 1:
                    eng = nc.sync if jc % 2 == 0 else nc.scalar
                    eng.dma_start(
                        xt[:], xs_d[:, jc * JC * T : (jc + 1) * JC * T]
                    )
                else:
                    cut = J7A * T
                    nc.sync.dma_start(
                        xt[:, :cut], xs_d[:, jc * JC * T : jc * JC * T + cut]
                    )
                    nc.scalar.dma_start(
                        xt[:, cut:], xs_d[:, jc * JC * T + cut : (jc + 1) * JC * T]
                    )
                for a in range(JC):
                    j = jc * JC + a
                    b, col = divmod(j * OUT_DIM, WHC)
                    nc.tensor.matmul(
                        psum_s[:],
                        wsts[b][:, col : col + OUT_DIM],
                        xt[:, a * T : (a + 1) * T],
                        start=(j == 0),
                        stop=(j == JW - 1),
                    )
                filler(TOPPER.get(jc, 0))

            # tail: d-fold via selector matmul, then the 10KB partial out
            s64_sb = aux.tile([OUT_DIM, T], F32R)
            with nc.allow_low_precision(
                reason="f32r rounding of s64 is the intended matmul precision"
            ):
                nc.vector.tensor_copy(s64_sb[:], psum_s[:])
            psum8 = pp.tile([HEADS, T], F32, name="psum8")
            nc.tensor.matmul(
                psum8[:],
                wst1[:OUT_DIM, WHC : WHC + HEADS],
                s64_sb[:],
                start=True,
                stop=True,
            )
            s_sbT = aux.tile([HEADS, T], F32)
            nc.vector.tensor_copy(s_sbT[:], psum8[:])
            nc.sync.dma_start(part_d[:], s_sbT[:])

    nc.compile()
    return nc


def build_fold():
    """Launch 2: core h folds head h's 8 partials, scales by adj^T, relu."""
    nc = bacc.Bacc("TRN2", debug=False, num_devices=NCORES, target_bir_lowering=False)

    # two parallel inputs (one per ring) so the ones-matmul only waits for
    # the 8 partial rows, not for adj^T
    finp_d = nc.dram_tensor("finp", [NCORES, T], F32R, kind="ExternalInput").ap()
    adjt_d = nc.dram_tensor("adjt", [NN, T], F32, kind="ExternalInput").ap()
    out_d = nc.dram_tensor("out", [NN, T], F32, kind="ExternalOutput").ap()

    with tile.TileContext(nc) as tc:
        with (
            tc.tile_pool(name="aux", bufs=1) as aux,
            tc.tile_pool(name="pp", bufs=1, space="PSUM") as pp,
        ):
            finp_sb = aux.tile([NCORES, T], F32R)
            nc.sync.dma_start(finp_sb[:], finp_d[:])
            adjt_sb = aux.tile([NN, T], F32)
            nc.scalar.dma_start(adjt_sb[:], adjt_d[:])
            # ones built on-device: no third DMA
            ones_sb = aux.tile([NCORES, NN], F32)
            nc.gpsimd.memset(ones_sb[:], 1.0)

            # ones-matmul: sums the 8 partial rows and replicates the sum
            # onto 10 PSUM partitions in one shot
            psum10 = pp.tile([NN, T], F32)
            nc.tensor.matmul(
                psum10[:],
                ones_sb[:].bitcast(F32R),
                finp_sb[:],
                start=True,
                stop=True,
            )
            # relu(s)*adj == relu(s*adj) since adj >= 0; one fused DVE op
            res = aux.tile([NN, T], F32)
            nc.vector.scalar_tensor_tensor(
                out=res[:],
                in0=psum10[:],
                scalar=0.0,
                in1=adjt_sb[:],
                op0=mybir.AluOpType.max,
                op1=mybir.AluOpType.mult,
            )
            nc.sync.dma_start(out_d[:], res[:])

    nc.compile()
    return nc


def shard_inputs(x, adj, W):
    """Host-side sharding/layout (pure data movement, no math)."""
    x2 = np.ascontiguousarray(x, dtype=np.float32).reshape(T, IN_DIM)
    # xs[c][p, j*T + t] = x2[t, c*KS + p*JW + j]
    xv = x2.reshape(T, NCORES, 128, JW).transpose(1, 2, 3, 0)  # (c, p, j, t)
    xs_all = np.ascontiguousarray(xv).reshape(NCORES, 128, JW * T)
    # ws[c][p, (j*8+h)*8+d] = W[h*8+d, c*KS + p*JW + j], then the selector
    # columns S[h*8+d, h'] = 1 iff h == h' (padded to 128 rows)
    Wv = np.ascontiguousarray(W, dtype=np.float32).reshape(HEADS, 8, NCORES, 128, JW)
    wv = Wv.transpose(2, 3, 4, 0, 1)  # (c, p, j, h, d)
    wcore = np.ascontiguousarray(wv).reshape(NCORES, 128, JW * OUT_DIM)
    sel = np.zeros((128, HEADS), dtype=np.float32)
    sel[:OUT_DIM] = np.kron(
        np.eye(HEADS, dtype=np.float32), np.ones((8, 1), dtype=np.float32)
    )
    ws_all = np.concatenate(
        [wcore, np.broadcast_to(sel, (NCORES, 128, HEADS))], axis=2
    )
    ws_all = np.ascontiguousarray(ws_all)
    return [{"xs": xs_all[c], "ws": ws_all[c]} for c in range(NCORES)]


_NC_MAIN = None
_NC_FOLD = None


def run(x, adj, W, trace=False, **kw):
    global _NC_MAIN, _NC_FOLD
    if _NC_MAIN is None:
        _NC_MAIN = build_main()
        _NC_FOLD = build_fold()

    res1 = run_bass_kernel_spmd(
        _NC_MAIN, shard_inputs(x, adj, W), core_ids=list(range(NCORES)),
        trace=trace, **kw
    )
    # host gather/scatter of the 10KB partials: core h gets row h of every
    # core's partial s^T (pure data movement)
    parts = np.stack([res1.results[c]["part"] for c in range(NCORES)])  # (c, h, t)
    adjt = np.ascontiguousarray(
        np.asarray(adj, dtype=np.float32).reshape(T, NN).T
    )
    in_maps2 = [
        {"finp": np.ascontiguousarray(parts[:, h, :]), "adjt": adjt}
        for h in range(HEADS)
    ]
    res2 = run_bass_kernel_spmd(
        _NC_FOLD, in_maps2, core_ids=list(range(NCORES)), trace=trace, **kw
    )

    full = np.empty((T, HEADS * NN), dtype=np.float32)
    for h in range(HEADS):
        full[:, h * NN : (h + 1) * NN] = res2.results[h]["out"].T
    return full.reshape(B, NN, HEADS * NN), (res1, res2)


def kernel(x, adj, W):
    out, _ = run(x, adj, W)
    return out
